# revision 29
# baseline (speedup 1.0000x reference)
"""MultiHeadAttention Trainium2 kernel.

Full inputs: x [4, 2048, 768] f32, W_qkv [2304, 768], W_proj [768, 768],
b_proj [768]. Output [4, 2048, 768] f32.

Sharding: 8 cores = 4 batches x 2 head-groups (6 heads each).
Per-core inputs (host-prepared, transposed on host):
  xT  [768, 2048]  = x[b].T
  wT  [768, 1152]  = concat(Wq_g, Wk_g, Wv_g).T   (g = head group rows)
  wpT [384, 768]   = W_proj[:, g-cols].T
Per-core output: outp [2048, 768] = partial projection output for batch b.
Host: out[b] = outp[2b] + outp[2b+1] + b_proj.

On-device (per core), ACT(exp)-bound software-pipelined stream:
  The softmax exp volume (6 heads x 2048^2 = 25.2M elems) pins the scalar
  engine near saturation (~207us at 1 elem/cycle + per-call overhead), so
  the kernel is one global stream of 192 "e-steps" (12 units = 3 head-pairs
  x 4 query-quarters, 16 key-blocks each); at step i the scalar engine runs
  exp(i) while the PE runs QK(i+2), AV(i) and the denominator pass:
  - QK energies: two heads computed CONCURRENTLY on the PE array via row
    tiling (K=64 each: head 2m on array rows 0-63, head 2m+1 on rows 64-127;
    tile_position auto-derived from base partitions).
  - AV: two heads concurrent via column tiling (M=64 each: outputs at psum
    partitions 0-63 / 64-127 of one bank).
  - softmax denominators: M=1 ones-matmuls run concurrently via column
    tiling; consecutive units use opposite halves of the denominator bank
    so its WAR never stalls.
  - PSUM (8 banks): e_ps 2x2 (double-buffered exp staging) + av 2 (bufs=2)
    + denom 1 + 1 filler bank.
  QKV projections for later pairs, v-blocks, and finished-quarter output
  projections are interleaved into PE gaps via a due-date-paced filler
  queue (half-blocks, <1us each); per-unit normalization (drain, one
  batched reciprocal, gpsimd broadcasts, bf16 multiplies) is deferred
  piecewise into the next unit's steps so its DVE burst never head-of-line
  blocks the filler psum bank.
"""

import ml_dtypes
import numpy as np

import concourse.bass as bass
import concourse.tile as tile
from concourse import bacc, mybir
from concourse.bass_utils import run_bass_kernel_spmd

EMB = 768
N = 2048
B = 4
D = 64
HL = 6            # heads per core
HD = HL * D       # 384 local head-dim columns
NCORES = 8
SCALE = D ** -0.5

F32 = mybir.dt.float32
BF16 = mybir.dt.bfloat16

EC = EMB // 128   # 6 emb chunks
MC = HD // 128    # 3 local head-dim chunks (= head pairs)
NQ = N // 512     # 4 query chunks of 512
NK = N // 128     # 16 key/seq chunks of 128

EXP = mybir.ActivationFunctionType.Exp


def _emit(tc):
    from contextlib import ExitStack

    nc = tc.nc
    xT = nc.dram_tensor("xT", [EMB, N], BF16, kind="ExternalInput").ap()
    wT = nc.dram_tensor("wT", [EMB, 3 * HD], BF16, kind="ExternalInput").ap()
    wpT = nc.dram_tensor("wpT", [HD, EMB], BF16, kind="ExternalInput").ap()
    outp = nc.dram_tensor("outp", [N, EMB], F32, kind="ExternalOutput").ap()

    xTr = xT.rearrange("(c p) s -> p c s", p=128)
    wTr = wT.rearrange("(c p) s -> p c s", p=128)
    wpTr = wpT.rearrange("(m p) e -> p m e", p=128)
    outr = outp.rearrange("(s p) e -> p s e", p=128)

    with ExitStack() as persist:
        ppool = persist.enter_context(tc.tile_pool(name="persist", bufs=1))
        # PE warmup junk matmuls run during the input-DMA wait and open the
        # HAM clock-gate before real work
        warm_sb = ppool.tile([128, 640], BF16)
        nc.vector.memset(warm_sb[:], 1.0)
        ones_sb = ppool.tile([128, 1], BF16)
        nc.vector.memset(ones_sb[:], 1.0)
        # preload the exp table set (~2.7us) during the input-DMA wait
        warm_act = ppool.tile([1, 16], BF16)
        nc.scalar.activation(warm_act[:], warm_sb[0:1, 0:16], EXP, scale=1.0)

        wp_sb = ppool.tile([128, MC, EMB], BF16)
        nc.sync.dma_start(wp_sb[:], wpTr)
        x_sb = ppool.tile([128, EC, N], BF16)
        w_sb = ppool.tile([128, EC, 3 * HD], BF16)
        for c in range(EC):
            nc.sync.dma_start(w_sb[:, c, :], wTr[:, c, :])
            nc.sync.dma_start(x_sb[:, c, :], xTr[:, c, :])

        # paired layouts: chunk m holds head 2m on partitions 0-63 and head
        # 2m+1 on partitions 64-127 (both q and k; v is [seq, 6*64])
        qT_sb = ppool.tile([128, MC, N], BF16)
        kT_sb = ppool.tile([128, MC, N], BF16)
        v_sb = ppool.tile([128, NK, HD], BF16)
        attT_sb = ppool.tile([128, MC, N], BF16)

        psum_pool = persist.enter_context(
            tc.tile_pool(name="psum", bufs=1, space="PSUM"))
        esb_pool = persist.enter_context(tc.tile_pool(name="esb", bufs=6))
        sm_pool = persist.enter_context(tc.tile_pool(name="sm", bufs=2))
        osb_pool = persist.enter_context(tc.tile_pool(name="osb", bufs=3))

        warm_ps = psum_pool.tile([128, 512], F32, tag="fill", bufs=1,
                                 name="warm_ps")
        for wi in range(16):
            nc.tensor.matmul(warm_ps[:], warm_sb[:, 0:128], warm_sb[:, 128:640],
                             start=(wi == 0), stop=(wi == 15))

        # ---------- phase-1 building blocks (also used as fillers) ----------
        def qk_block(which, m, n, tag="fill", bufs=1):
            # qT/kT chunk m, q-block n: psum [128, 512] accumulated over EC
            lo = which * HD + m * 128
            ns = slice(n * 512, (n + 1) * 512)
            mm = psum_pool.tile([128, 512], F32, tag=tag, bufs=bufs,
                                name=f"mm_{which}_{m}_{n}")
            for c in range(EC):
                nc.tensor.matmul(mm[:], w_sb[:, c, lo:lo + 128],
                                 x_sb[:, c, ns],
                                 start=(c == 0), stop=(c == EC - 1))
            dst = qT_sb if which == 0 else kT_sb
            nc.vector.tensor_copy(dst[:, m, ns], mm[:])

        def v_block(p, s, tag="fill", bufs=1):
            # v for pair p, seq block s
            vv = psum_pool.tile([128, 512], F32, tag=tag, bufs=bufs,
                                name=f"vv_{p}_{s}")[:, 0:128]
            for c in range(EC):
                nc.tensor.matmul(
                    vv[:],
                    x_sb[:, c, s * 128:(s + 1) * 128],
                    w_sb[:, c, 2 * HD + p * 128:2 * HD + (p + 1) * 128],
                    start=(c == 0), stop=(c == EC - 1))
            nc.vector.tensor_copy(v_sb[:, s, p * 128:(p + 1) * 128], vv[:])

        def proj_block(s):
            # out[s-block, :] = sum_m attT[:, m, sblk].T @ wp[m]
            o_sb = osb_pool.tile([128, EMB], F32, tag="osb", name=f"osb_{s}")
            ss = slice(s * 128, (s + 1) * 128)
            for half in range(2):
                pr = psum_pool.tile([128, 512], F32, tag="fill", bufs=1,
                                    name=f"pr_{s}_{half}")[:, 0:HD]
                for m in range(MC):
                    nc.tensor.matmul(
                        pr[:], attT_sb[:, m, ss],
                        wp_sb[:, m, half * HD:(half + 1) * HD],
                        start=(m == 0), stop=(m == MC - 1))
                nc.vector.tensor_copy(o_sb[:, half * HD:(half + 1) * HD],
                                      pr[:])
            nc.sync.dma_start(outr[:, s, :], o_sb[:])

        # ---------- startup: minimum work to begin pair-0 attention ----------
        # attention psum banks are still free; use them for 2-deep pipelining
        for n in range(NQ):
            qk_block(1, 0, n, tag="av", bufs=2)  # kT chunk 0 (kk=step now)
        qk_block(0, 0, 0, tag="av", bufs=2)      # qT for quarter 0
        for s in range(4):
            v_block(0, s, tag="dps" if s % 2 else "fill", bufs=1)

        # filler queue: (due_step, emit_fn) pumped under the ACT stream.
        # qk blocks are split into halves so one pump stays ~<1us of PE.
        fillq = []

        def add_qk(which, m, n, due):
            state = {}
            lo = which * HD + m * 128
            ns = slice(n * 512, (n + 1) * 512)

            def half_a():
                state["mm"] = psum_pool.tile([128, 512], F32, tag="fill",
                                             bufs=1, name=f"mm_{which}_{m}_{n}")
                for c in range(3):
                    nc.tensor.matmul(state["mm"][:], w_sb[:, c, lo:lo + 128],
                                     x_sb[:, c, ns], start=(c == 0), stop=False)

            def half_b():
                for c in range(3, EC):
                    nc.tensor.matmul(state["mm"][:], w_sb[:, c, lo:lo + 128],
                                     x_sb[:, c, ns], start=False,
                                     stop=(c == EC - 1))
                dst = qT_sb if which == 0 else kT_sb
                nc.vector.tensor_copy(dst[:, m, ns], state["mm"][:])

            fillq.append((due, half_a))
            fillq.append((due, half_b))

        for j in range(1, 4):                        # qT(0,j): quarter j
            add_qk(0, 0, j, 16 * j - 4)
        for s in range(4, NK):
            fillq.append((s - 2, lambda s=s: v_block(0, s)))
        for n in range(NQ):                          # kT(1,n)
            add_qk(1, 1, n, 58 + 4 * n)
        for j in range(4):                           # qT(1,j)
            add_qk(0, 1, j, 58 + 16 * j)
        for s in range(NK):
            fillq.append((58 + s, lambda s=s: v_block(1, s)))
        for n in range(NQ):                          # kT(2,n)
            add_qk(1, 2, n, 122 + 4 * n)
        for j in range(4):                           # qT(2,j)
            add_qk(0, 2, j, 122 + 16 * j)
        for s in range(NK):
            fillq.append((110 + s, lambda s=s: v_block(2, s)))
        fillq.sort(key=lambda t: t[0])

        def add_proj(s):
            state = {}
            ss = slice(s * 128, (s + 1) * 128)

            def half(hf):
                def go():
                    if hf == 0:
                        state["o"] = osb_pool.tile([128, EMB], F32, tag="osb",
                                                   name=f"osb_{s}")
                    pr = psum_pool.tile([128, 512], F32, tag="fill", bufs=1,
                                        name=f"pr_{s}_{hf}")[:, 0:HD]
                    for m in range(MC):
                        nc.tensor.matmul(
                            pr[:], attT_sb[:, m, ss],
                            wp_sb[:, m, hf * HD:(hf + 1) * HD],
                            start=(m == 0), stop=(m == MC - 1))
                    nc.vector.tensor_copy(
                        state["o"][:, hf * HD:(hf + 1) * HD], pr[:])
                    if hf == 1:
                        nc.sync.dma_start(outr[:, s, :], state["o"][:])
                return go

            fillq.append((10 ** 9, half(0)))
            fillq.append((10 ** 9, half(1)))

        # ---------- attention: ACT-bound software-pipelined stream ----------
        # units: (m, first 512-q-block, #q-blocks); m=2 runs quarter-units so
        # the final normalize + projection tail is short
        units = [(m, j, 1) for m in range(MC) for j in range(4)]
        steps, sin_unit = [], []
        for ui, (m, qb0, nb) in enumerate(units):
            for kk in range(NK):
                for qb in range(nb):
                    sin_unit.append(len(steps) - (len(sin_unit) - sin_unit[-1] - 1 if sin_unit else 0))
                    steps.append((ui, kk, qb))
        # recompute step-in-unit cleanly
        sin_unit = []
        for ui, (m, qb0, nb) in enumerate(units):
            sin_unit.extend(range(NK * nb))

        unit_state = {}
        norm_q = []  # deferred normalize pieces, popped one per step

        def get_unit(ui):
            if ui not in unit_state:
                m, qb0, nb = units[ui]
                avs = [psum_pool.tile([128, 512], F32, tag="av", bufs=2,
                                      name=f"av_{ui}_{qb}")
                       for qb in range(nb)]
                d_ps = psum_pool.tile([128, 512], F32, tag="dps", bufs=1,
                                      name=f"dps_{ui}")
                unit_state[ui] = (avs, d_ps, {})
            return unit_state[ui]

        def emit_qk(step):
            ui, kk, qb = step
            m, qb0, nb = units[ui]
            ks = slice(kk * 128, (kk + 1) * 128)
            qs = slice((qb0 + qb) * 512, (qb0 + qb + 1) * 512)
            eps = psum_pool.tile([128, 2, 512], F32, tag="eps", bufs=2,
                                 name=f"eps_{ui}_{kk}_{qb}")
            # two heads concurrently: row tiles (0,0) and (64,0)
            nc.tensor.matmul(eps[:, 0, :], kT_sb[0:64, m, ks],
                             qT_sb[0:64, m, qs], start=True, stop=True)
            nc.tensor.matmul(eps[:, 1, :], kT_sb[64:128, m, ks],
                             qT_sb[64:128, m, qs], start=True, stop=True)
            return eps

        def queue_normalize(ui, avs, d_ps):
            # drains now (free psum fast); the slow reciprocal/broadcast/mult
            # chain is deferred into the next unit's steps, piecewise
            m, qb0, nb = units[ui]
            avst = sm_pool.tile([128, 512], BF16, tag="avst", bufs=6,
                                name=f"avst_{ui}")
            nc.vector.tensor_copy(avst[:], avs[0][:])
            ca = sm_pool.tile([64, 512], F32, tag="dsb", bufs=3,
                              name=f"ca_{ui}")
            nc.vector.tensor_copy(ca[:], d_ps[0:64, :])
            cb = sm_pool.tile([64, 512], F32, tag="dsb2", bufs=3,
                              name=f"cb_{ui}")
            nc.vector.tensor_copy(cb[:], d_ps[64:128, :])
            state = {}

            def recip():
                # sum the kk-parity halves; one reciprocal covers both
                # denominator rows (DVE cost scales with free dim only)
                ds = sm_pool.tile([64, 512], F32, tag="dsum", bufs=3,
                                  name=f"ds_{ui}")
                nc.vector.tensor_add(ds[:], ca[:], cb[:])
                state["dre"] = sm_pool.tile([64, 512], F32, tag="dre",
                                            bufs=3, name=f"dre_{ui}")
                nc.vector.reciprocal(state["dre"][:], ds[:])

            def bcast(h):
                def go():
                    rec = sm_pool.tile([1, 512], BF16, tag="rec", bufs=12,
                                       name=f"rec_{ui}_{h}")
                    nc.vector.tensor_copy(rec[:], state["dre"][h * 32:h * 32 + 1, :])
                    rb = sm_pool.tile([128, 512], BF16, tag="rb", bufs=12,
                                      name=f"rb_{ui}_{h}")
                    nc.gpsimd.partition_broadcast(rb[:], rec[:])
                    state[h] = rb
                return go

            def mult(h, is_last):
                def go():
                    qs = slice(qb0 * 512, (qb0 + 1) * 512)
                    nc.vector.tensor_mul(
                        attT_sb[h * 64:h * 64 + 64, m, qs],
                        avst[h * 64:h * 64 + 64, :],
                        state[h][h * 64:h * 64 + 64, :])
                    if is_last and m == 2 and qb0 < 3:
                        # this quarter's attT is complete: its four output
                        # s-blocks can now project as fillers
                        for s in range(4 * qb0, 4 * qb0 + 4):
                            add_proj(s)
                return go

            norm_q.append(recip)
            for h in range(2):
                norm_q.append(bcast(h))
            for h in range(2):
                norm_q.append(mult(h, h == 1))

        def pump_step(i):
            # forced: everything due within the 2-step QK lookahead
            while fillq and fillq[0][0] <= i + 2:
                fillq.pop(0)[1]()
            # optional: paced, avoiding the first steps of a unit where the
            # previous unit's drains still occupy the DVE queue
            if fillq and (i < 32 or sin_unit[i] >= 5):
                fillq.pop(0)[1]()

        eps_of = {0: emit_qk(steps[0]), 1: emit_qk(steps[1])}

        for i, step in enumerate(steps):
            ui, kk, qb = step
            m, qb0, nb = units[ui]
            avs, d_ps, e_sbs = get_unit(ui)
            first, last = kk == 0, kk == NK - 1
            e_sb = esb_pool.tile([128, 2, 512], BF16, tag="esb",
                                 name=f"esb_{ui}_{kk}_{qb}")
            nc.scalar.activation(e_sb[:], eps_of.pop(i)[:], EXP, scale=SCALE)
            e_sbs[kk % 2] = e_sb
            if i + 2 < len(steps):
                eps_of[i + 2] = emit_qk(steps[i + 2])
            # AV pair: col tiles (0,0) and (0,64) concurrent
            nc.tensor.matmul(
                avs[qb][0:64, :],
                v_sb[:, kk, (2 * m) * D:(2 * m) * D + D],
                e_sb[:, 0, :], start=first, stop=last)
            nc.tensor.matmul(
                avs[qb][64:128, :],
                v_sb[:, kk, (2 * m + 1) * D:(2 * m + 1) * D + D],
                e_sb[:, 1, :], start=first, stop=last)
            if kk % 2 == 1:
                # denominators: one span of 4 concurrent M=1 col tiles per 2
                # steps; even-kk terms accumulate at rows 0/32, odd-kk terms
                # at rows 64/96 (halves summed before the reciprocal)
                for par in range(2):
                    for h in range(2):
                        r = par * 64 + h * 32
                        nc.tensor.matmul(d_ps[r:r + 1, :], ones_sb[:],
                                         e_sbs[par][:, h, :],
                                         start=(kk == 1), stop=last,
                                         tile_position=(0, r))
            if last and qb == nb - 1:
                queue_normalize(ui, avs, d_ps)
                del unit_state[ui]
            pump_step(i)
            # normalize pieces go AFTER the filler so the filler's psum-
            # freeing copy sits ahead of them in the DVE FIFO
            if sin_unit[i] >= 3 or i < 16:
                for _ in range(2):
                    if norm_q:
                        norm_q.pop(0)()

        # ---------- tail: last quarter normalize + remaining projection ----
        while norm_q:
            norm_q.pop(0)()
        while fillq:
            fillq.pop(0)[1]()
        # last quarter's projections pipeline through the free att banks
        tags = [("av", 2), ("dps", 1), ("fill", 1)]
        for j, s in enumerate(range(12, NK)):
            tag, bufs = tags[j % 3]
            o_sb = osb_pool.tile([128, EMB], F32, tag="osb", name=f"osb_{s}")
            ss = slice(s * 128, (s + 1) * 128)
            for half in range(2):
                pr = psum_pool.tile([128, 512], F32, tag=tag, bufs=bufs,
                                    name=f"pr_{s}_{half}")[:, 0:HD]
                for m in range(MC):
                    nc.tensor.matmul(
                        pr[:], attT_sb[:, m, ss],
                        wp_sb[:, m, half * HD:(half + 1) * HD],
                        start=(m == 0), stop=(m == MC - 1))
                nc.vector.tensor_copy(o_sb[:, half * HD:(half + 1) * HD],
                                      pr[:])
            nc.sync.dma_start(outr[:, s, :], o_sb[:])


_CACHE = {}


def _build():
    if "nc" not in _CACHE:
        nc = bacc.Bacc("TRN2", target_bir_lowering=False, debug=False,
                       num_devices=NCORES)
        with tile.TileContext(nc) as tc:
            _emit(tc)
        nc.compile()
        _CACHE["nc"] = nc
    return _CACHE["nc"]


def _in_maps(x, W_qkv, W_proj):
    in_maps = []
    for c in range(NCORES):
        b, g = divmod(c, 2)
        r0 = g * HD
        w_rows = np.concatenate([
            W_qkv[0 * EMB + r0: 0 * EMB + r0 + HD],
            W_qkv[1 * EMB + r0: 1 * EMB + r0 + HD],
            W_qkv[2 * EMB + r0: 2 * EMB + r0 + HD],
        ], axis=0)                                   # [1152, 768]
        bf = ml_dtypes.bfloat16
        in_maps.append({
            "xT": np.ascontiguousarray(x[b].T.astype(bf)),
            "wT": np.ascontiguousarray(w_rows.T.astype(bf)),
            "wpT": np.ascontiguousarray(W_proj[:, r0:r0 + HD].T.astype(bf)),
        })
    return in_maps


LAST_RESULTS = None


def kernel(x, W_qkv, W_proj, b_proj):
    global LAST_RESULTS
    x = np.ascontiguousarray(np.asarray(x, dtype=np.float32))
    W_qkv = np.asarray(W_qkv, dtype=np.float32)
    W_proj = np.asarray(W_proj, dtype=np.float32)
    b_proj = np.asarray(b_proj, dtype=np.float32)

    nc = _build()
    in_maps = _in_maps(x, W_qkv, W_proj)
    res = run_bass_kernel_spmd(nc, in_maps, core_ids=list(range(NCORES)))
    LAST_RESULTS = res

    out = np.empty((B, N, EMB), dtype=np.float32)
    for b in range(B):
        out[b] = res.results[2 * b]["outp"] + res.results[2 * b + 1]["outp"]
    out += b_proj
    return out



# revision 30
# speedup vs baseline: 1.0555x; 1.0555x over previous
"""MultiHeadAttention Trainium2 kernel.

Full inputs: x [4, 2048, 768] f32, W_qkv [2304, 768], W_proj [768, 768],
b_proj [768]. Output [4, 2048, 768] f32.

Sharding: 8 cores = 4 batches x 2 head-groups (6 heads each).
Per-core inputs (host-prepared, transposed on host):
  xT  [768, 2048]  = x[b].T
  wT  [768, 1152]  = concat(Wq_g, Wk_g, Wv_g).T   (g = head group rows)
  wpT [384, 768]   = W_proj[:, g-cols].T
Per-core output: outp [2048, 768] = partial projection output for batch b.
Host: out[b] = outp[2b] + outp[2b+1] + b_proj.

On-device (per core), ACT(exp)-bound software-pipelined stream:
  The softmax exp volume (6 heads x 2048^2 = 25.2M elems) pins the scalar
  engine near saturation (~207us at 1 elem/cycle + per-call overhead), so
  the kernel is one global stream of 192 "e-steps" (12 units = 3 head-pairs
  x 4 query-quarters, 16 key-blocks each); at step i the scalar engine runs
  exp(i) while the PE runs QK(i+2), AV(i) and the denominator pass:
  - QK energies: two heads computed CONCURRENTLY on the PE array via row
    tiling (K=64 each: head 2m on array rows 0-63, head 2m+1 on rows 64-127;
    tile_position auto-derived from base partitions).
  - AV: two heads concurrent via column tiling (M=64 each: outputs at psum
    partitions 0-63 / 64-127 of one bank).
  - softmax denominators: M=1 ones-matmuls run concurrently via column
    tiling; consecutive units use opposite halves of the denominator bank
    so its WAR never stalls.
  - PSUM (8 banks): e_ps 2x2 (double-buffered exp staging) + av 2 (bufs=2)
    + denom 1 + 1 filler bank.
  QKV projections for later pairs, v-blocks, and finished-quarter output
  projections are interleaved into PE gaps via a due-date-paced filler
  queue (half-blocks, <1us each); per-unit normalization (drain, one
  batched reciprocal, gpsimd broadcasts, bf16 multiplies) is deferred
  piecewise into the next unit's steps so its DVE burst never head-of-line
  blocks the filler psum bank.
"""

import ml_dtypes
import numpy as np

import concourse.bass as bass
import concourse.tile as tile
from concourse import bacc, mybir
from concourse.bass_utils import run_bass_kernel_spmd

EMB = 768
N = 2048
B = 4
D = 64
HL = 6            # heads per core
HD = HL * D       # 384 local head-dim columns
NCORES = 8
SCALE = D ** -0.5

F32 = mybir.dt.float32
BF16 = mybir.dt.bfloat16

EC = EMB // 128   # 6 emb chunks
MC = HD // 128    # 3 local head-dim chunks (= head pairs)
NQ = N // 512     # 4 query chunks of 512
NK = N // 128     # 16 key/seq chunks of 128

EXP = mybir.ActivationFunctionType.Exp


def _emit(tc):
    from contextlib import ExitStack

    nc = tc.nc
    xT = nc.dram_tensor("xT", [EMB, N], BF16, kind="ExternalInput").ap()
    wT = nc.dram_tensor("wT", [EMB, 3 * HD], BF16, kind="ExternalInput").ap()
    wpT = nc.dram_tensor("wpT", [HD, EMB], BF16, kind="ExternalInput").ap()
    outp = nc.dram_tensor("outp", [N, EMB], F32, kind="ExternalOutput").ap()

    xTr = xT.rearrange("(c p) s -> p c s", p=128)
    wTr = wT.rearrange("(c p) s -> p c s", p=128)
    wpTr = wpT.rearrange("(m p) e -> p m e", p=128)
    outr = outp.rearrange("(s p) e -> p s e", p=128)

    with ExitStack() as persist:
        ppool = persist.enter_context(tc.tile_pool(name="persist", bufs=1))
        # PE warmup junk matmuls run during the input-DMA wait and open the
        # HAM clock-gate before real work
        warm_sb = ppool.tile([128, 640], BF16)
        nc.vector.memset(warm_sb[:], 1.0)
        ones_sb = ppool.tile([128, 1], BF16)
        nc.vector.memset(ones_sb[:], 1.0)
        # preload the exp table set (~2.7us) during the input-DMA wait
        warm_act = ppool.tile([1, 16], BF16)
        nc.scalar.activation(warm_act[:], warm_sb[0:1, 0:16], EXP, scale=1.0)

        wp_sb = ppool.tile([128, MC, EMB], BF16)
        nc.sync.dma_start(wp_sb[:], wpTr)
        x_sb = ppool.tile([128, EC, N], BF16)
        w_sb = ppool.tile([128, EC, 3 * HD], BF16)
        for c in range(EC):
            nc.sync.dma_start(w_sb[:, c, :], wTr[:, c, :])
            nc.sync.dma_start(x_sb[:, c, :], xTr[:, c, :])

        # paired layouts: chunk m holds head 2m on partitions 0-63 and head
        # 2m+1 on partitions 64-127 (both q and k; v is [seq, 6*64])
        qT_sb = ppool.tile([128, MC, N], BF16)
        kT_sb = ppool.tile([128, MC, N], BF16)
        v_sb = ppool.tile([128, NK, HD], BF16)
        attT_sb = ppool.tile([128, MC, N], BF16)

        psum_pool = persist.enter_context(
            tc.tile_pool(name="psum", bufs=1, space="PSUM"))
        esb_pool = persist.enter_context(tc.tile_pool(name="esb", bufs=4))
        sm_pool = persist.enter_context(tc.tile_pool(name="sm", bufs=2))
        osb_pool = persist.enter_context(tc.tile_pool(name="osb", bufs=3))

        warm_ps = psum_pool.tile([128, 512], F32, tag="fill", bufs=1,
                                 name="warm_ps")
        for wi in range(16):
            nc.tensor.matmul(warm_ps[:], warm_sb[:, 0:128], warm_sb[:, 128:640],
                             start=(wi == 0), stop=(wi == 15))

        # ---------- phase-1 building blocks (also used as fillers) ----------
        def qk_block(which, m, n, tag="fill", bufs=1):
            # qT/kT chunk m, q-block n: psum [128, 512] accumulated over EC
            lo = which * HD + m * 128
            ns = slice(n * 512, (n + 1) * 512)
            mm = psum_pool.tile([128, 512], F32, tag=tag, bufs=bufs,
                                name=f"mm_{which}_{m}_{n}")
            for c in range(EC):
                nc.tensor.matmul(mm[:], w_sb[:, c, lo:lo + 128],
                                 x_sb[:, c, ns],
                                 start=(c == 0), stop=(c == EC - 1))
            dst = qT_sb if which == 0 else kT_sb
            nc.vector.tensor_copy(dst[:, m, ns], mm[:])

        def v_block(p, s, tag="fill", bufs=1):
            # v for pair p, seq block s
            vv = psum_pool.tile([128, 512], F32, tag=tag, bufs=bufs,
                                name=f"vv_{p}_{s}")[:, 0:128]
            for c in range(EC):
                nc.tensor.matmul(
                    vv[:],
                    x_sb[:, c, s * 128:(s + 1) * 128],
                    w_sb[:, c, 2 * HD + p * 128:2 * HD + (p + 1) * 128],
                    start=(c == 0), stop=(c == EC - 1))
            nc.vector.tensor_copy(v_sb[:, s, p * 128:(p + 1) * 128], vv[:])

        def proj_block(s):
            # out[s-block, :] = sum_m attT[:, m, sblk].T @ wp[m]
            o_sb = osb_pool.tile([128, EMB], F32, tag="osb", name=f"osb_{s}")
            ss = slice(s * 128, (s + 1) * 128)
            for half in range(2):
                pr = psum_pool.tile([128, 512], F32, tag="fill", bufs=1,
                                    name=f"pr_{s}_{half}")[:, 0:HD]
                for m in range(MC):
                    nc.tensor.matmul(
                        pr[:], attT_sb[:, m, ss],
                        wp_sb[:, m, half * HD:(half + 1) * HD],
                        start=(m == 0), stop=(m == MC - 1))
                nc.vector.tensor_copy(o_sb[:, half * HD:(half + 1) * HD],
                                      pr[:])
            nc.sync.dma_start(outr[:, s, :], o_sb[:])

        # ---------- startup: minimum work to begin pair-0 attention ----------
        # attention psum banks are still free; use them for 2-deep pipelining
        for n in range(NQ):
            qk_block(1, 0, n, tag="av", bufs=2)  # kT chunk 0 (kk=step now)
        qk_block(0, 0, 0, tag="av", bufs=2)      # qT for quarter 0
        for s in range(4):
            v_block(0, s, tag="dps" if s % 2 else "fill", bufs=1)

        # filler queue: (due_step, emit_fn) pumped under the ACT stream.
        # qk blocks are split into halves so one pump stays ~<1us of PE.
        fillq = []

        def add_qk(which, m, n, due):
            state = {}
            lo = which * HD + m * 128
            ns = slice(n * 512, (n + 1) * 512)

            def half_a():
                state["mm"] = psum_pool.tile([128, 512], F32, tag="fill",
                                             bufs=1, name=f"mm_{which}_{m}_{n}")
                for c in range(3):
                    nc.tensor.matmul(state["mm"][:], w_sb[:, c, lo:lo + 128],
                                     x_sb[:, c, ns], start=(c == 0), stop=False)

            def half_b():
                for c in range(3, EC):
                    nc.tensor.matmul(state["mm"][:], w_sb[:, c, lo:lo + 128],
                                     x_sb[:, c, ns], start=False,
                                     stop=(c == EC - 1))
                dst = qT_sb if which == 0 else kT_sb
                nc.vector.tensor_copy(dst[:, m, ns], state["mm"][:])

            fillq.append((due, half_a))
            fillq.append((due, half_b))

        for j in range(1, 4):                        # qT(0,j): quarter j
            add_qk(0, 0, j, 16 * j - 4)
        for s in range(4, NK):
            fillq.append((s - 2, lambda s=s: v_block(0, s)))
        for n in range(NQ):                          # kT(1,n)
            add_qk(1, 1, n, 58 + 4 * n)
        for j in range(4):                           # qT(1,j)
            add_qk(0, 1, j, 58 + 16 * j)
        for s in range(NK):
            fillq.append((54 + s, lambda s=s: v_block(1, s)))
        for n in range(NQ):                          # kT(2,n)
            add_qk(1, 2, n, 122 + 4 * n)
        for j in range(4):                           # qT(2,j)
            add_qk(0, 2, j, 122 + 16 * j)
        for s in range(NK):
            fillq.append((118 + s, lambda s=s: v_block(2, s)))
        fillq.sort(key=lambda t: t[0])

        def add_proj(s):
            state = {}
            ss = slice(s * 128, (s + 1) * 128)

            def half(hf):
                def go():
                    if hf == 0:
                        state["o"] = osb_pool.tile([128, EMB], F32, tag="osb",
                                                   name=f"osb_{s}")
                    pr = psum_pool.tile([128, 512], F32, tag="fill", bufs=1,
                                        name=f"pr_{s}_{hf}")[:, 0:HD]
                    for m in range(MC):
                        nc.tensor.matmul(
                            pr[:], attT_sb[:, m, ss],
                            wp_sb[:, m, hf * HD:(hf + 1) * HD],
                            start=(m == 0), stop=(m == MC - 1))
                    nc.vector.tensor_copy(
                        state["o"][:, hf * HD:(hf + 1) * HD], pr[:])
                    if hf == 1:
                        nc.sync.dma_start(outr[:, s, :], state["o"][:])
                return go

            fillq.append((10 ** 9, half(0)))
            fillq.append((10 ** 9, half(1)))

        # ---------- attention: ACT-bound software-pipelined stream ----------
        # units: (m, first 512-q-block, #q-blocks); m=2 runs quarter-units so
        # the final normalize + projection tail is short
        units = [(m, j, 1) for m in range(MC) for j in range(4)]
        steps, sin_unit = [], []
        for ui, (m, qb0, nb) in enumerate(units):
            for kk in range(NK):
                for qb in range(nb):
                    sin_unit.append(len(steps) - (len(sin_unit) - sin_unit[-1] - 1 if sin_unit else 0))
                    steps.append((ui, kk, qb))
        # recompute step-in-unit cleanly
        sin_unit = []
        for ui, (m, qb0, nb) in enumerate(units):
            sin_unit.extend(range(NK * nb))

        unit_state = {}
        norm_q = []  # deferred normalize pieces, popped one per step

        def get_unit(ui):
            if ui not in unit_state:
                m, qb0, nb = units[ui]
                avs = [psum_pool.tile([128, 512], F32, tag="av", bufs=2,
                                      name=f"av_{ui}_{qb}")
                       for qb in range(nb)]
                d_ps = psum_pool.tile([128, 512], F32, tag="dps", bufs=1,
                                      name=f"dps_{ui}")
                unit_state[ui] = (avs, d_ps, {})
            return unit_state[ui]

        def emit_qk(step):
            ui, kk, qb = step
            m, qb0, nb = units[ui]
            ks = slice(kk * 128, (kk + 1) * 128)
            qs = slice((qb0 + qb) * 512, (qb0 + qb + 1) * 512)
            eps = psum_pool.tile([128, 2, 512], F32, tag="eps", bufs=2,
                                 name=f"eps_{ui}_{kk}_{qb}")
            # two heads concurrently: row tiles (0,0) and (64,0)
            nc.tensor.matmul(eps[:, 0, :], kT_sb[0:64, m, ks],
                             qT_sb[0:64, m, qs], start=True, stop=True)
            nc.tensor.matmul(eps[:, 1, :], kT_sb[64:128, m, ks],
                             qT_sb[64:128, m, qs], start=True, stop=True)
            return eps

        def queue_normalize(ui, avs, d_ps):
            # drains now (free psum fast); the slow reciprocal/broadcast/mult
            # chain is deferred into the next unit's steps, piecewise
            m, qb0, nb = units[ui]
            avst = sm_pool.tile([128, 512], BF16, tag="avst", bufs=6,
                                name=f"avst_{ui}")
            nc.vector.tensor_copy(avst[:], avs[0][:])
            ca = sm_pool.tile([64, 512], F32, tag="dsb", bufs=3,
                              name=f"ca_{ui}")
            nc.vector.tensor_copy(ca[:], d_ps[0:64, :])
            cb = sm_pool.tile([64, 512], F32, tag="dsb2", bufs=3,
                              name=f"cb_{ui}")
            nc.vector.tensor_copy(cb[:], d_ps[64:128, :])
            state = {}

            def recip():
                # sum the kk-parity halves; one reciprocal covers both
                # denominator rows (DVE cost scales with free dim only)
                ds = sm_pool.tile([64, 512], F32, tag="dsum", bufs=3,
                                  name=f"ds_{ui}")
                nc.vector.tensor_add(ds[:], ca[:], cb[:])
                state["dre"] = sm_pool.tile([64, 512], F32, tag="dre",
                                            bufs=3, name=f"dre_{ui}")
                nc.vector.reciprocal(state["dre"][:], ds[:])

            def bcast(h):
                def go():
                    rec = sm_pool.tile([1, 512], BF16, tag="rec", bufs=12,
                                       name=f"rec_{ui}_{h}")
                    nc.vector.tensor_copy(rec[:], state["dre"][h * 32:h * 32 + 1, :])
                    rb = sm_pool.tile([128, 512], BF16, tag="rb", bufs=12,
                                      name=f"rb_{ui}_{h}")
                    nc.gpsimd.partition_broadcast(rb[:], rec[:])
                    state[h] = rb
                return go

            def mult(h, is_last):
                def go():
                    qs = slice(qb0 * 512, (qb0 + 1) * 512)
                    nc.vector.tensor_mul(
                        attT_sb[h * 64:h * 64 + 64, m, qs],
                        avst[h * 64:h * 64 + 64, :],
                        state[h][h * 64:h * 64 + 64, :])
                    if is_last and m == 2 and qb0 < 3:
                        # this quarter's attT is complete: its four output
                        # s-blocks can now project as fillers
                        for s in range(4 * qb0, 4 * qb0 + 4):
                            add_proj(s)
                return go

            norm_q.append(recip)
            for h in range(2):
                norm_q.append(bcast(h))
            for h in range(2):
                norm_q.append(mult(h, h == 1))

        def pump_step(i):
            # forced: everything due within the 2-step QK lookahead
            while fillq and fillq[0][0] <= i + 2:
                fillq.pop(0)[1]()
            # optional: paced, avoiding the first steps of a unit where the
            # previous unit's drains still occupy the DVE queue
            if fillq and (i < 32 or sin_unit[i] >= 6):
                fillq.pop(0)[1]()

        eps_of = {0: emit_qk(steps[0]), 1: emit_qk(steps[1])}

        for i, step in enumerate(steps):
            ui, kk, qb = step
            m, qb0, nb = units[ui]
            avs, d_ps, e_sbs = get_unit(ui)
            first, last = kk == 0, kk == NK - 1
            e_sb = esb_pool.tile([128, 2, 512], BF16, tag="esb",
                                 name=f"esb_{ui}_{kk}_{qb}")
            nc.scalar.activation(e_sb[:], eps_of.pop(i)[:], EXP, scale=SCALE)
            e_sbs[kk % 2] = e_sb
            if i + 2 < len(steps):
                eps_of[i + 2] = emit_qk(steps[i + 2])
            # AV pair: col tiles (0,0) and (0,64) concurrent
            nc.tensor.matmul(
                avs[qb][0:64, :],
                v_sb[:, kk, (2 * m) * D:(2 * m) * D + D],
                e_sb[:, 0, :], start=first, stop=last)
            nc.tensor.matmul(
                avs[qb][64:128, :],
                v_sb[:, kk, (2 * m + 1) * D:(2 * m + 1) * D + D],
                e_sb[:, 1, :], start=first, stop=last)
            if kk % 2 == 1:
                # denominators: one span of 4 concurrent M=1 col tiles per 2
                # steps; even-kk terms accumulate at rows 0/32, odd-kk terms
                # at rows 64/96 (halves summed before the reciprocal)
                for par in range(2):
                    for h in range(2):
                        r = par * 64 + h * 32
                        nc.tensor.matmul(d_ps[r:r + 1, :], ones_sb[:],
                                         e_sbs[par][:, h, :],
                                         start=(kk == 1), stop=last,
                                         tile_position=(0, r))
            if last and qb == nb - 1:
                queue_normalize(ui, avs, d_ps)
                del unit_state[ui]
            pump_step(i)
            # normalize pieces go AFTER the filler so the filler's psum-
            # freeing copy sits ahead of them in the DVE FIFO
            if sin_unit[i] >= 3 or i < 16:
                for _ in range(2):
                    if norm_q:
                        norm_q.pop(0)()

        # ---------- tail: last quarter normalize + remaining projection ----
        while norm_q:
            norm_q.pop(0)()
        while fillq:
            fillq.pop(0)[1]()
        # last quarter's projections pipeline through the free att banks
        tags = [("av", 2), ("dps", 1), ("fill", 1)]
        for j, s in enumerate(range(12, NK)):
            tag, bufs = tags[j % 3]
            o_sb = osb_pool.tile([128, EMB], F32, tag="osb", name=f"osb_{s}")
            ss = slice(s * 128, (s + 1) * 128)
            for half in range(2):
                pr = psum_pool.tile([128, 512], F32, tag=tag, bufs=bufs,
                                    name=f"pr_{s}_{half}")[:, 0:HD]
                for m in range(MC):
                    nc.tensor.matmul(
                        pr[:], attT_sb[:, m, ss],
                        wp_sb[:, m, half * HD:(half + 1) * HD],
                        start=(m == 0), stop=(m == MC - 1))
                nc.vector.tensor_copy(o_sb[:, half * HD:(half + 1) * HD],
                                      pr[:])
            nc.sync.dma_start(outr[:, s, :], o_sb[:])


_CACHE = {}


def _build():
    if "nc" not in _CACHE:
        nc = bacc.Bacc("TRN2", target_bir_lowering=False, debug=False,
                       num_devices=NCORES)
        with tile.TileContext(nc) as tc:
            _emit(tc)
        nc.compile()
        _CACHE["nc"] = nc
    return _CACHE["nc"]


def _in_maps(x, W_qkv, W_proj):
    in_maps = []
    for c in range(NCORES):
        b, g = divmod(c, 2)
        r0 = g * HD
        w_rows = np.concatenate([
            W_qkv[0 * EMB + r0: 0 * EMB + r0 + HD],
            W_qkv[1 * EMB + r0: 1 * EMB + r0 + HD],
            W_qkv[2 * EMB + r0: 2 * EMB + r0 + HD],
        ], axis=0)                                   # [1152, 768]
        bf = ml_dtypes.bfloat16
        in_maps.append({
            "xT": np.ascontiguousarray(x[b].T.astype(bf)),
            "wT": np.ascontiguousarray(w_rows.T.astype(bf)),
            "wpT": np.ascontiguousarray(W_proj[:, r0:r0 + HD].T.astype(bf)),
        })
    return in_maps


LAST_RESULTS = None


def kernel(x, W_qkv, W_proj, b_proj):
    global LAST_RESULTS
    x = np.ascontiguousarray(np.asarray(x, dtype=np.float32))
    W_qkv = np.asarray(W_qkv, dtype=np.float32)
    W_proj = np.asarray(W_proj, dtype=np.float32)
    b_proj = np.asarray(b_proj, dtype=np.float32)

    nc = _build()
    in_maps = _in_maps(x, W_qkv, W_proj)
    res = run_bass_kernel_spmd(nc, in_maps, core_ids=list(range(NCORES)))
    LAST_RESULTS = res

    out = np.empty((B, N, EMB), dtype=np.float32)
    for b in range(B):
        out[b] = res.results[2 * b]["outp"] + res.results[2 * b + 1]["outp"]
    out += b_proj
    return out



# revision 32
# speedup vs baseline: 1.0668x; 1.0107x over previous
"""MultiHeadAttention Trainium2 kernel.

Full inputs: x [4, 2048, 768] f32, W_qkv [2304, 768], W_proj [768, 768],
b_proj [768]. Output [4, 2048, 768] f32.

Sharding: 8 cores = 4 batches x 2 head-groups (6 heads each).
Per-core inputs (host-prepared, transposed on host):
  xT  [768, 2048]  = x[b].T
  wT  [768, 1152]  = concat(Wq_g, Wk_g, Wv_g).T   (g = head group rows)
  wpT [384, 768]   = W_proj[:, g-cols].T
Per-core output: outp [2048, 768] = partial projection output for batch b.
Host: out[b] = outp[2b] + outp[2b+1] + b_proj.

On-device (per core), ACT(exp)-bound software-pipelined stream:
  The softmax exp volume (6 heads x 2048^2 = 25.2M elems) pins the scalar
  engine near saturation (~207us at 1 elem/cycle + per-call overhead), so
  the kernel is one global stream of 192 "e-steps" (12 units = 3 head-pairs
  x 4 query-quarters, 16 key-blocks each); at step i the scalar engine runs
  exp(i) while the PE runs QK(i+2), AV(i) and the denominator pass:
  - QK energies: two heads computed CONCURRENTLY on the PE array via row
    tiling (K=64 each: head 2m on array rows 0-63, head 2m+1 on rows 64-127;
    tile_position auto-derived from base partitions).
  - AV: two heads concurrent via column tiling (M=64 each: outputs at psum
    partitions 0-63 / 64-127 of one bank).
  - softmax denominators: M=1 ones-matmuls via column tiling; even-kk
    terms accumulate at bank rows 0/32 and odd-kk terms at rows 64/96, so
    all four run in ONE concurrent PE span every other step (halves are
    summed on the DVE before the reciprocal).
  - PSUM (8 banks): e_ps 2x2 (double-buffered exp staging) + av 2 (bufs=2)
    + denom 1 + 1 filler bank.
  QKV projections for later pairs, v-blocks, and finished-quarter output
  projections are interleaved into PE gaps via a due-date-paced filler
  queue (half-blocks, <1us each); per-unit normalization (drain, one
  batched reciprocal, gpsimd broadcasts, bf16 multiplies) is deferred
  piecewise into the next unit's steps so its DVE burst never head-of-line
  blocks the filler psum bank.
"""

import ml_dtypes
import numpy as np

import concourse.bass as bass
import concourse.tile as tile
from concourse import bacc, mybir
from concourse.bass_utils import run_bass_kernel_spmd

EMB = 768
N = 2048
B = 4
D = 64
HL = 6            # heads per core
HD = HL * D       # 384 local head-dim columns
NCORES = 8
SCALE = D ** -0.5

F32 = mybir.dt.float32
BF16 = mybir.dt.bfloat16

EC = EMB // 128   # 6 emb chunks
MC = HD // 128    # 3 local head-dim chunks (= head pairs)
NQ = N // 512     # 4 query chunks of 512
NK = N // 128     # 16 key/seq chunks of 128

EXP = mybir.ActivationFunctionType.Exp


def _emit(tc):
    from contextlib import ExitStack

    nc = tc.nc
    xT = nc.dram_tensor("xT", [EMB, N], BF16, kind="ExternalInput").ap()
    wT = nc.dram_tensor("wT", [EMB, 3 * HD], BF16, kind="ExternalInput").ap()
    wpT = nc.dram_tensor("wpT", [HD, EMB], BF16, kind="ExternalInput").ap()
    outp = nc.dram_tensor("outp", [N, EMB], F32, kind="ExternalOutput").ap()

    xTr = xT.rearrange("(c p) s -> p c s", p=128)
    wTr = wT.rearrange("(c p) s -> p c s", p=128)
    wpTr = wpT.rearrange("(m p) e -> p m e", p=128)
    outr = outp.rearrange("(s p) e -> p s e", p=128)

    with ExitStack() as persist:
        ppool = persist.enter_context(tc.tile_pool(name="persist", bufs=1))
        # PE warmup junk matmuls run during the input-DMA wait and open the
        # HAM clock-gate before real work
        warm_sb = ppool.tile([128, 640], BF16)
        nc.vector.memset(warm_sb[:], 1.0)
        ones_sb = ppool.tile([128, 1], BF16)
        nc.vector.memset(ones_sb[:], 1.0)
        # preload the exp table set (~2.7us) during the input-DMA wait
        warm_act = ppool.tile([1, 16], BF16)
        nc.scalar.activation(warm_act[:], warm_sb[0:1, 0:16], EXP, scale=1.0)

        wp_sb = ppool.tile([128, MC, EMB], BF16)
        x_sb = ppool.tile([128, EC, N], BF16)
        w_sb = ppool.tile([128, EC, 3 * HD], BF16)
        for c in range(EC):
            nc.sync.dma_start(w_sb[:, c, :], wTr[:, c, :])
            nc.sync.dma_start(x_sb[:, c, :], xTr[:, c, :])
        # needed only by the projection ~150us later: keep it off the
        # startup critical path
        nc.sync.dma_start(wp_sb[:], wpTr)

        # paired layouts: chunk m holds head 2m on partitions 0-63 and head
        # 2m+1 on partitions 64-127 (both q and k; v is [seq, 6*64])
        qT_sb = ppool.tile([128, MC, N], BF16)
        kT_sb = ppool.tile([128, MC, N], BF16)
        v_sb = ppool.tile([128, NK, HD], BF16)
        attT_sb = ppool.tile([128, MC, N], BF16)

        psum_pool = persist.enter_context(
            tc.tile_pool(name="psum", bufs=1, space="PSUM"))
        esb_pool = persist.enter_context(tc.tile_pool(name="esb", bufs=4))
        sm_pool = persist.enter_context(tc.tile_pool(name="sm", bufs=2))
        osb_pool = persist.enter_context(tc.tile_pool(name="osb", bufs=3))

        warm_ps = psum_pool.tile([128, 512], F32, tag="fill", bufs=1,
                                 name="warm_ps")
        for wi in range(16):
            nc.tensor.matmul(warm_ps[:], warm_sb[:, 0:128], warm_sb[:, 128:640],
                             start=(wi == 0), stop=(wi == 15))

        # ---------- phase-1 building blocks (also used as fillers) ----------
        def qk_block(which, m, n, tag="fill", bufs=1):
            # qT/kT chunk m, q-block n: psum [128, 512] accumulated over EC
            lo = which * HD + m * 128
            ns = slice(n * 512, (n + 1) * 512)
            mm = psum_pool.tile([128, 512], F32, tag=tag, bufs=bufs,
                                name=f"mm_{which}_{m}_{n}")
            for c in range(EC):
                nc.tensor.matmul(mm[:], w_sb[:, c, lo:lo + 128],
                                 x_sb[:, c, ns],
                                 start=(c == 0), stop=(c == EC - 1))
            dst = qT_sb if which == 0 else kT_sb
            nc.vector.tensor_copy(dst[:, m, ns], mm[:])

        def v_block(p, s, tag="fill", bufs=1):
            # v for pair p, seq block s
            vv = psum_pool.tile([128, 512], F32, tag=tag, bufs=bufs,
                                name=f"vv_{p}_{s}")[:, 0:128]
            for c in range(EC):
                nc.tensor.matmul(
                    vv[:],
                    x_sb[:, c, s * 128:(s + 1) * 128],
                    w_sb[:, c, 2 * HD + p * 128:2 * HD + (p + 1) * 128],
                    start=(c == 0), stop=(c == EC - 1))
            nc.vector.tensor_copy(v_sb[:, s, p * 128:(p + 1) * 128], vv[:])

        def proj_block(s):
            # out[s-block, :] = sum_m attT[:, m, sblk].T @ wp[m]
            o_sb = osb_pool.tile([128, EMB], F32, tag="osb", name=f"osb_{s}")
            ss = slice(s * 128, (s + 1) * 128)
            for half in range(2):
                pr = psum_pool.tile([128, 512], F32, tag="fill", bufs=1,
                                    name=f"pr_{s}_{half}")[:, 0:HD]
                for m in range(MC):
                    nc.tensor.matmul(
                        pr[:], attT_sb[:, m, ss],
                        wp_sb[:, m, half * HD:(half + 1) * HD],
                        start=(m == 0), stop=(m == MC - 1))
                nc.vector.tensor_copy(o_sb[:, half * HD:(half + 1) * HD],
                                      pr[:])
            nc.sync.dma_start(outr[:, s, :], o_sb[:])

        # ---------- startup: minimum work to begin pair-0 attention ----------
        # attention psum banks are still free; use them for 2-deep pipelining
        for n in range(NQ):
            qk_block(1, 0, n, tag="av", bufs=2)  # kT chunk 0 (kk=step now)
        qk_block(0, 0, 0, tag="av", bufs=2)      # qT for quarter 0
        for s in range(4):
            v_block(0, s, tag="dps" if s % 2 else "fill", bufs=1)

        # filler queue: (due_step, emit_fn) pumped under the ACT stream.
        # qk blocks are split into halves so one pump stays ~<1us of PE.
        fillq = []

        def add_qk(which, m, n, due):
            state = {}
            lo = which * HD + m * 128
            ns = slice(n * 512, (n + 1) * 512)

            def half_a():
                state["mm"] = psum_pool.tile([128, 512], F32, tag="fill",
                                             bufs=1, name=f"mm_{which}_{m}_{n}")
                for c in range(3):
                    nc.tensor.matmul(state["mm"][:], w_sb[:, c, lo:lo + 128],
                                     x_sb[:, c, ns], start=(c == 0), stop=False)

            def half_b():
                for c in range(3, EC):
                    nc.tensor.matmul(state["mm"][:], w_sb[:, c, lo:lo + 128],
                                     x_sb[:, c, ns], start=False,
                                     stop=(c == EC - 1))
                dst = qT_sb if which == 0 else kT_sb
                nc.vector.tensor_copy(dst[:, m, ns], state["mm"][:])

            fillq.append((due, half_a))
            fillq.append((due, half_b))

        for j in range(1, 4):                        # qT(0,j): quarter j
            add_qk(0, 0, j, 16 * j - 4)
        for s in range(4, NK):
            fillq.append((s - 2, lambda s=s: v_block(0, s)))
        for n in range(NQ):                          # kT(1,n)
            add_qk(1, 1, n, 58 + 4 * n)
        for j in range(4):                           # qT(1,j)
            add_qk(0, 1, j, 58 + 16 * j)
        for s in range(NK):
            fillq.append((54 + s, lambda s=s: v_block(1, s)))
        for n in range(NQ):                          # kT(2,n)
            add_qk(1, 2, n, 122 + 4 * n)
        for j in range(4):                           # qT(2,j)
            add_qk(0, 2, j, 122 + 16 * j)
        for s in range(NK):
            fillq.append((118 + s, lambda s=s: v_block(2, s)))
        fillq.sort(key=lambda t: t[0])

        def add_proj(s):
            state = {}
            ss = slice(s * 128, (s + 1) * 128)

            def half(hf):
                def go():
                    if hf == 0:
                        state["o"] = osb_pool.tile([128, EMB], F32, tag="osb",
                                                   name=f"osb_{s}")
                    pr = psum_pool.tile([128, 512], F32, tag="fill", bufs=1,
                                        name=f"pr_{s}_{hf}")[:, 0:HD]
                    for m in range(MC):
                        nc.tensor.matmul(
                            pr[:], attT_sb[:, m, ss],
                            wp_sb[:, m, hf * HD:(hf + 1) * HD],
                            start=(m == 0), stop=(m == MC - 1))
                    nc.vector.tensor_copy(
                        state["o"][:, hf * HD:(hf + 1) * HD], pr[:])
                    if hf == 1:
                        nc.sync.dma_start(outr[:, s, :], state["o"][:])
                return go

            fillq.append((10 ** 9, half(0)))
            fillq.append((10 ** 9, half(1)))

        # ---------- attention: ACT-bound software-pipelined stream ----------
        # units: (m, first 512-q-block, #q-blocks); m=2 runs quarter-units so
        # the final normalize + projection tail is short
        units = [(m, j, 1) for m in range(MC) for j in range(4)]
        steps, sin_unit = [], []
        for ui, (m, qb0, nb) in enumerate(units):
            for kk in range(NK):
                for qb in range(nb):
                    sin_unit.append(len(steps) - (len(sin_unit) - sin_unit[-1] - 1 if sin_unit else 0))
                    steps.append((ui, kk, qb))
        # recompute step-in-unit cleanly
        sin_unit = []
        for ui, (m, qb0, nb) in enumerate(units):
            sin_unit.extend(range(NK * nb))

        unit_state = {}
        norm_q = []  # deferred normalize pieces, popped one per step

        def get_unit(ui):
            if ui not in unit_state:
                m, qb0, nb = units[ui]
                avs = [psum_pool.tile([128, 512], F32, tag="av", bufs=2,
                                      name=f"av_{ui}_{qb}")
                       for qb in range(nb)]
                d_ps = psum_pool.tile([128, 512], F32, tag="dps", bufs=1,
                                      name=f"dps_{ui}")
                unit_state[ui] = (avs, d_ps, {})
            return unit_state[ui]

        def emit_qk(step):
            ui, kk, qb = step
            m, qb0, nb = units[ui]
            ks = slice(kk * 128, (kk + 1) * 128)
            qs = slice((qb0 + qb) * 512, (qb0 + qb + 1) * 512)
            eps = psum_pool.tile([128, 2, 512], F32, tag="eps", bufs=2,
                                 name=f"eps_{ui}_{kk}_{qb}")
            # two heads concurrently: row tiles (0,0) and (64,0)
            nc.tensor.matmul(eps[:, 0, :], kT_sb[0:64, m, ks],
                             qT_sb[0:64, m, qs], start=True, stop=True)
            nc.tensor.matmul(eps[:, 1, :], kT_sb[64:128, m, ks],
                             qT_sb[64:128, m, qs], start=True, stop=True)
            return eps

        def queue_normalize(ui, avs, d_ps):
            # drains now (free psum fast); the slow reciprocal/broadcast/mult
            # chain is deferred into the next unit's steps, piecewise
            m, qb0, nb = units[ui]
            avst = sm_pool.tile([128, 512], BF16, tag="avst", bufs=6,
                                name=f"avst_{ui}")
            nc.vector.tensor_copy(avst[:], avs[0][:])
            ca = sm_pool.tile([64, 512], F32, tag="dsb", bufs=3,
                              name=f"ca_{ui}")
            nc.vector.tensor_copy(ca[:], d_ps[0:64, :])
            cb = sm_pool.tile([64, 512], F32, tag="dsb2", bufs=3,
                              name=f"cb_{ui}")
            nc.vector.tensor_copy(cb[:], d_ps[64:128, :])
            state = {}

            def recip():
                # sum the kk-parity halves; one reciprocal covers both
                # denominator rows (DVE cost scales with free dim only)
                ds = sm_pool.tile([64, 512], F32, tag="dsum", bufs=3,
                                  name=f"ds_{ui}")
                nc.vector.tensor_add(ds[:], ca[:], cb[:])
                state["dre"] = sm_pool.tile([64, 512], F32, tag="dre",
                                            bufs=3, name=f"dre_{ui}")
                nc.vector.reciprocal(state["dre"][:], ds[:])

            def bcast(h):
                def go():
                    rec = sm_pool.tile([1, 512], BF16, tag="rec", bufs=12,
                                       name=f"rec_{ui}_{h}")
                    nc.vector.tensor_copy(rec[:], state["dre"][h * 32:h * 32 + 1, :])
                    rb = sm_pool.tile([128, 512], BF16, tag="rb", bufs=12,
                                      name=f"rb_{ui}_{h}")
                    nc.gpsimd.partition_broadcast(rb[:], rec[:])
                    state[h] = rb
                return go

            def mult(h, is_last):
                def go():
                    qs = slice(qb0 * 512, (qb0 + 1) * 512)
                    nc.vector.tensor_mul(
                        attT_sb[h * 64:h * 64 + 64, m, qs],
                        avst[h * 64:h * 64 + 64, :],
                        state[h][h * 64:h * 64 + 64, :])
                    if is_last and m == 2 and qb0 < 2:
                        # this quarter's attT is complete: its four output
                        # s-blocks can now project as fillers
                        for s in range(4 * qb0, 4 * qb0 + 4):
                            add_proj(s)
                return go

            norm_q.append(recip)
            for h in range(2):
                norm_q.append(bcast(h))
            for h in range(2):
                norm_q.append(mult(h, h == 1))

        def pump_step(i):
            # forced: everything due within the 2-step QK lookahead
            while fillq and fillq[0][0] <= i + 2:
                fillq.pop(0)[1]()
            # optional: paced, avoiding the first steps of a unit where the
            # previous unit's drains still occupy the DVE queue
            if fillq and (i < 32 or sin_unit[i] >= 6):
                fillq.pop(0)[1]()

        eps_of = {0: emit_qk(steps[0]), 1: emit_qk(steps[1])}

        for i, step in enumerate(steps):
            ui, kk, qb = step
            m, qb0, nb = units[ui]
            avs, d_ps, e_sbs = get_unit(ui)
            first, last = kk == 0, kk == NK - 1
            e_sb = esb_pool.tile([128, 2, 512], BF16, tag="esb",
                                 name=f"esb_{ui}_{kk}_{qb}")
            nc.scalar.activation(e_sb[:], eps_of.pop(i)[:], EXP, scale=SCALE)
            e_sbs[kk % 2] = e_sb
            if i + 2 < len(steps):
                eps_of[i + 2] = emit_qk(steps[i + 2])
            # AV pair: col tiles (0,0) and (0,64) concurrent
            nc.tensor.matmul(
                avs[qb][0:64, :],
                v_sb[:, kk, (2 * m) * D:(2 * m) * D + D],
                e_sb[:, 0, :], start=first, stop=last)
            nc.tensor.matmul(
                avs[qb][64:128, :],
                v_sb[:, kk, (2 * m + 1) * D:(2 * m + 1) * D + D],
                e_sb[:, 1, :], start=first, stop=last)
            if kk % 2 == 1:
                # denominators: one span of 4 concurrent M=1 col tiles per 2
                # steps; even-kk terms accumulate at rows 0/32, odd-kk terms
                # at rows 64/96 (halves summed before the reciprocal)
                for par in range(2):
                    for h in range(2):
                        r = par * 64 + h * 32
                        nc.tensor.matmul(d_ps[r:r + 1, :], ones_sb[:],
                                         e_sbs[par][:, h, :],
                                         start=(kk == 1), stop=last,
                                         tile_position=(0, r))
            if last and qb == nb - 1:
                queue_normalize(ui, avs, d_ps)
                del unit_state[ui]
            pump_step(i)
            # normalize pieces go AFTER the filler so the filler's psum-
            # freeing copy sits ahead of them in the DVE FIFO
            if sin_unit[i] >= 3 or i < 16:
                for _ in range(2):
                    if norm_q:
                        norm_q.pop(0)()

        # ---------- tail: last quarter normalize + remaining projection ----
        while norm_q:
            norm_q.pop(0)()
        while fillq:
            fillq.pop(0)[1]()
        # last quarter's projections pipeline through the free att banks
        tags = [("av", 2), ("dps", 1), ("fill", 1)]
        for j, s in enumerate(range(8, NK)):
            tag, bufs = tags[j % 3]
            o_sb = osb_pool.tile([128, EMB], F32, tag="osb", name=f"osb_{s}")
            ss = slice(s * 128, (s + 1) * 128)
            for half in range(2):
                pr = psum_pool.tile([128, 512], F32, tag=tag, bufs=bufs,
                                    name=f"pr_{s}_{half}")[:, 0:HD]
                for m in range(MC):
                    nc.tensor.matmul(
                        pr[:], attT_sb[:, m, ss],
                        wp_sb[:, m, half * HD:(half + 1) * HD],
                        start=(m == 0), stop=(m == MC - 1))
                nc.vector.tensor_copy(o_sb[:, half * HD:(half + 1) * HD],
                                      pr[:])
            nc.sync.dma_start(outr[:, s, :], o_sb[:])


_CACHE = {}


def _build():
    if "nc" not in _CACHE:
        nc = bacc.Bacc("TRN2", target_bir_lowering=False, debug=False,
                       num_devices=NCORES)
        with tile.TileContext(nc) as tc:
            _emit(tc)
        nc.compile()
        _CACHE["nc"] = nc
    return _CACHE["nc"]


def _in_maps(x, W_qkv, W_proj):
    in_maps = []
    for c in range(NCORES):
        b, g = divmod(c, 2)
        r0 = g * HD
        w_rows = np.concatenate([
            W_qkv[0 * EMB + r0: 0 * EMB + r0 + HD],
            W_qkv[1 * EMB + r0: 1 * EMB + r0 + HD],
            W_qkv[2 * EMB + r0: 2 * EMB + r0 + HD],
        ], axis=0)                                   # [1152, 768]
        bf = ml_dtypes.bfloat16
        in_maps.append({
            "xT": np.ascontiguousarray(x[b].T.astype(bf)),
            "wT": np.ascontiguousarray(w_rows.T.astype(bf)),
            "wpT": np.ascontiguousarray(W_proj[:, r0:r0 + HD].T.astype(bf)),
        })
    return in_maps


LAST_RESULTS = None


def kernel(x, W_qkv, W_proj, b_proj):
    global LAST_RESULTS
    x = np.ascontiguousarray(np.asarray(x, dtype=np.float32))
    W_qkv = np.asarray(W_qkv, dtype=np.float32)
    W_proj = np.asarray(W_proj, dtype=np.float32)
    b_proj = np.asarray(b_proj, dtype=np.float32)

    nc = _build()
    in_maps = _in_maps(x, W_qkv, W_proj)
    res = run_bass_kernel_spmd(nc, in_maps, core_ids=list(range(NCORES)))
    LAST_RESULTS = res

    out = np.empty((B, N, EMB), dtype=np.float32)
    for b in range(B):
        out[b] = res.results[2 * b]["outp"] + res.results[2 * b + 1]["outp"]
    out += b_proj
    return out

